# revision 4
# baseline (speedup 1.0000x reference)
"""AdaptiveRotatedScaledConv2d on 8 TRN2 NeuronCores.

Strategy (data-parallel over batch, 2 samples per core):
  - Host: build per-sample 9x9 bilinear rotation/scale matrices (tiny),
    transform the shared weight bank into per-sample 3x3 kernels
    (one sgemm, ~0.7 GFLOP), lay out as matmul-ready [Cin, tap, Cout].
  - Device: per-sample conv as 18 accumulating TensorE matmuls per
    512-wide output tile (9 taps x 2 Cin blocks), operating on shifted
    views of x held in SBUF with zero margins for the H boundary and
    column-zeroed x variants for the W boundary.
  - Compute dtype bf16 (f32 PSUM accumulation), output f32.
"""

import os

import ml_dtypes
import numpy as np

import concourse.mybir as mybir
from concourse import bacc
from concourse.bass_utils import run_bass_kernel_spmd
from concourse.tile import TileContext

B, Cin, Cout, H, W, NK = 16, 256, 256, 64, 64, 4
NCORES = 8
SPC = B // NCORES          # samples per core
HW = H * W                 # 4096
MARGIN = 65                # covers max shift |dy*W + dx| = 65
XBUF = HW + 2 * MARGIN     # 4226
CBLK = 128
NCB = Cin // CBLK          # 2 Cin partition blocks
NOB = Cout // CBLK         # 2 Cout partition blocks
TFREE = 512                # matmul free-dim tile (one PSUM bank of f32)
NT = HW // TFREE           # 8 output tiles per sample
# x variant by kx (kernel column): kx=0 reads col x-1 -> needs col 63
# zeroed (wrap from previous row); kx=2 reads col x+1 -> needs col 0
# zeroed; kx=1 reads in-place.
VARIANT_BY_KX = {0: 2, 1: 0, 2: 1}  # v0 orig, v1 zero-col0, v2 zero-col63

TRACE = bool(int(os.environ.get("KERNEL_TRACE", "0")))
LAST_RESULT = None  # stash of BassKernelResults for test harness


def _ensure_ntff_hook():
    """The RL container's `antenv` stub lacks `axon_hooks`; provide it and
    register the ctypes NTFF profile hook so trace=True yields exec_time_ns."""
    import sys
    import types

    import concourse.bass_utils as bu

    # Keep profiling artifacts local; no bucket in the sandbox.
    bu.upload_artifacts = lambda tmpdir: tmpdir
    try:
        import antenv.axon_hooks  # noqa: F401
        return
    except ImportError:
        pass
    import antenv

    hook = {"h": None}
    so_path = os.environ.get("PJRT_LIBRARY_PATH")
    if so_path and os.path.exists(so_path):
        try:
            from trn_agent_boot.trn_boot import _ntff_profile_via_ctypes
            hook["h"] = _ntff_profile_via_ctypes(so_path)
        except Exception as e:  # pragma: no cover
            print(f"ntff hook setup failed: {e}")
    mod = types.ModuleType("antenv.axon_hooks")
    mod.get_axon_ntff_profile_hook = lambda: hook["h"]
    mod.set_axon_ntff_profile_hook = lambda h: hook.update(h=h)
    sys.modules["antenv.axon_hooks"] = mod
    antenv.axon_hooks = mod


def _rot_mat_np(thetas, scales):
    """Numpy port of reference._rot_mat: [bs, g] -> [bs, g, 9, 9]."""
    bs, g = thetas.shape
    t = thetas.reshape(-1)
    s = scales.reshape(-1)
    x = np.cos(t) * s
    y = np.sin(t) * s
    yp = -y
    z = np.zeros_like(x)
    o = np.ones_like(x)
    a = x - y; b = x * y; c = x + y; d = a * c; e = a + c
    ap = x - yp; bp = x * yp; cp = x + yp; dp = ap * cp; ep = ap + cp

    def M(rows):
        return np.stack([np.stack(r, axis=0) for r in rows], axis=0)

    ctr = [z, z, z, z, o, z, z, z, z]

    pb1 = M([
        [a, 1 - a, z, z, z, z, z, z, z],
        [z, 1 - y, y, z, z, z, z, z, z],
        [z, z, a, z, z, 1 - a, z, z, z],
        [y, z, z, 1 - y, z, z, z, z, z],
        ctr,
        [z, z, z, z, z, 1 - y, z, z, y],
        [z, z, z, 1 - a, z, z, a, z, z],
        [z, z, z, z, z, z, y, 1 - y, z],
        [z, z, z, z, z, z, z, 1 - a, a]])
    pb2 = M([
        [a, 1 - a, z, z, z, z, z, z, z],
        [z, x - b, b, z, 1 - c + b, y - b, z, z, z],
        [z, z, a, z, z, 1 - a, z, z, z],
        [b, y - b, z, x - b, 1 - c + b, z, z, z, z],
        ctr,
        [z, z, z, z, 1 - c + b, x - b, z, y - b, b],
        [z, z, z, 1 - a, z, z, a, z, z],
        [z, z, z, y - b, 1 - c + b, z, b, x - b, z],
        [z, z, z, z, z, z, z, 1 - a, a]])
    ps1 = M([
        [d, a - d, z, c - d, 1 - e + d, z, z, z, z],
        [z, x - b, b, z, 1 - c + b, y - b, z, z, z],
        [z, c - d, d, z, 1 - e + d, a - d, z, z, z],
        [b, y - b, z, x - b, 1 - c + b, z, z, z, z],
        ctr,
        [z, z, z, z, 1 - c + b, x - b, z, y - b, b],
        [z, z, z, a - d, 1 - e + d, z, d, c - d, z],
        [z, z, z, y - b, 1 - c + b, z, b, x - b, z],
        [z, z, z, z, 1 - e + d, c - d, z, a - d, d]])
    ps2 = pb2
    nb1 = M([
        [cp, z, z, 1 - cp, z, z, z, z, z],
        [yp, 1 - yp, z, z, z, z, z, z, z],
        [z, 1 - cp, cp, z, z, z, z, z, z],
        [z, z, z, 1 - yp, z, z, yp, z, z],
        ctr,
        [z, z, yp, z, z, 1 - yp, z, z, z],
        [z, z, z, z, z, z, cp, 1 - cp, z],
        [z, z, z, z, z, z, z, 1 - yp, yp],
        [z, z, z, z, z, 1 - cp, z, z, cp]])
    nb2 = M([
        [cp, z, z, 1 - cp, z, z, z, z, z],
        [bp, x - bp, z, yp - bp, 1 - cp + bp, z, z, z, z],
        [z, 1 - cp, cp, z, z, z, z, z, z],
        [z, z, z, x - bp, 1 - cp + bp, z, bp, yp - bp, z],
        ctr,
        [z, yp - bp, bp, z, 1 - cp + bp, x - bp, z, z, z],
        [z, z, z, z, z, z, cp, 1 - cp, z],
        [z, z, z, z, 1 - cp + bp, yp - bp, z, x - bp, bp],
        [z, z, z, z, z, 1 - cp, z, z, cp]])
    ns1 = M([
        [dp, cp - dp, z, ap - dp, 1 - ep + dp, z, z, z, z],
        [bp, x - bp, z, yp - bp, 1 - cp + bp, z, z, z, z],
        [z, ap - dp, dp, z, 1 - ep + dp, cp - dp, z, z, z],
        [z, yp - bp, bp, z, 1 - cp + bp, x - bp, z, z, z],
        ctr,
        [z, z, z, x - bp, 1 - cp + bp, z, bp, yp - bp, z],
        [z, z, z, cp - dp, 1 - ep + dp, z, dp, ap - dp, z],
        [z, z, z, z, 1 - cp + bp, yp - bp, z, x - bp, bp],
        [z, z, z, z, 1 - ep + dp, ap - dp, z, cp - dp, dp]])
    ns2 = nb2

    m_pos = (t >= 0.0)[None, None, :]
    m_big = (s >= 1.0)[None, None, :]
    m_1 = (np.abs(t) <= np.pi / 4)[None, None, :]
    pos = np.where(m_big, np.where(m_1, pb1, pb2), np.where(m_1, ps1, ps2))
    neg = np.where(m_big, np.where(m_1, nb1, nb2), np.where(m_1, ns1, ns2))
    rot = np.where(m_pos, pos, neg)  # [9, 9, bs*g]
    return rot.transpose(2, 0, 1).reshape(bs, g, 9, 9)


def _transform_weights(thetas, scales, lambdas, weight):
    """-> per-sample kernels w[b, i(tap), o, c], float32."""
    rot = _rot_mat_np(thetas, scales) * lambdas[:, :, None, None]  # [B,n,9,9]
    # w[b,i,o,c] = sum_{n,j} rot[b,n,i,j] * w9[n,o,c,j]
    R = rot.transpose(0, 2, 1, 3).reshape(B * 9, NK * 9)           # [(b i),(n j)]
    W9 = weight.reshape(NK, Cout, Cin, 9).transpose(0, 3, 1, 2)    # [n,j,o,c]
    W9 = np.ascontiguousarray(W9).reshape(NK * 9, Cout * Cin)
    return (R @ W9).reshape(B, 9, Cout, Cin)


def _build_graph():
    bf16 = mybir.dt.bfloat16
    f32 = mybir.dt.float32
    nc = bacc.Bacc(None, target_bir_lowering=False)
    xv_ext = nc.declare_dram_parameter(
        "xv", [3, SPC, NCB, CBLK, HW], bf16, isOutput=False)
    wt_ext = nc.declare_dram_parameter(
        "wt", [SPC, NCB, CBLK, 9 * Cout], bf16, isOutput=False)
    out_ext = nc.declare_dram_parameter(
        "out", [SPC, NOB, CBLK, HW], f32, isOutput=True)

    with TileContext(nc) as tc:
        with (
            tc.tile_pool(name="xpool", bufs=1) as xpool,
            tc.tile_pool(name="wpool", bufs=1) as wpool,
            tc.tile_pool(name="opool", bufs=4) as opool,
            tc.tile_pool(name="ppool", bufs=4, space="PSUM") as ppool,
        ):
            xsb = {}
            for s in range(SPC):
                for cb in range(NCB):
                    for v in range(3):
                        t = xpool.tile([CBLK, XBUF], bf16, tag=f"x{s}{cb}{v}")
                        nc.vector.memset(t[:, 0:MARGIN], 0.0)
                        nc.vector.memset(t[:, MARGIN + HW:XBUF], 0.0)
                        nc.sync.dma_start(
                            out=t[:, MARGIN:MARGIN + HW], in_=xv_ext[v, s, cb])
                        xsb[(s, cb, v)] = t
            wsb = {}
            for s in range(SPC):
                for cb in range(NCB):
                    t = wpool.tile([CBLK, 9 * Cout], bf16, tag=f"w{s}{cb}")
                    nc.sync.dma_start(out=t[:], in_=wt_ext[s, cb])
                    wsb[(s, cb)] = t

            for s in range(SPC):
                for ob in range(NOB):
                    for ti in range(NT):
                        ps = ppool.tile([CBLK, TFREE], f32)
                        k = 0
                        for cb in range(NCB):
                            for tap in range(9):
                                ky, kx = tap // 3, tap % 3
                                off = (ky - 1) * W + (kx - 1)
                                v = VARIANT_BY_KX[kx]
                                base = MARGIN + ti * TFREE + off
                                rhs = xsb[(s, cb, v)][:, base:base + TFREE]
                                lhsT = wsb[(s, cb)][
                                    :, tap * Cout + ob * CBLK:
                                       tap * Cout + ob * CBLK + CBLK]
                                nc.tensor.matmul(
                                    ps[:], lhsT, rhs,
                                    start=(k == 0), stop=(k == 2 * 9 - 1))
                                k += 1
                        ot = opool.tile([CBLK, TFREE], f32)
                        nc.vector.tensor_copy(out=ot[:], in_=ps[:])
                        nc.sync.dma_start(
                            out=out_ext[s, ob, :, ti * TFREE:(ti + 1) * TFREE],
                            in_=ot[:])
    nc.compile()
    return nc


def kernel(x, thetas, scales, lambdas, weight):
    global LAST_RESULT
    x = np.asarray(x, dtype=np.float32)
    thetas = np.asarray(thetas, dtype=np.float32)
    scales = np.asarray(scales, dtype=np.float32)
    lambdas = np.asarray(lambdas, dtype=np.float32)
    weight = np.asarray(weight, dtype=np.float32)

    # Host: per-sample transformed kernels, matmul-ready layout.
    wt = _transform_weights(thetas, scales, lambdas, weight)  # [B,9,Cout,Cin]
    wt = wt.transpose(0, 3, 1, 2)                             # [B,Cin,9,Cout]
    wt = np.ascontiguousarray(wt).reshape(B, NCB, CBLK, 9 * Cout)
    wt = wt.astype(ml_dtypes.bfloat16)

    # Host: x variants for the W-boundary (see VARIANT_BY_KX).
    xb = x.astype(ml_dtypes.bfloat16).reshape(B, NCB, CBLK, H, W)
    xv = np.empty((3, B, NCB, CBLK, H, W), dtype=ml_dtypes.bfloat16)
    xv[0] = xb
    xv[1] = xb
    xv[1][..., 0] = 0
    xv[2] = xb
    xv[2][..., W - 1] = 0
    xv = xv.reshape(3, B, NCB, CBLK, HW)

    if TRACE:
        _ensure_ntff_hook()
    nc = _build_graph()
    in_maps = []
    for c in range(NCORES):
        sl = slice(c * SPC, (c + 1) * SPC)
        in_maps.append({
            "xv": np.ascontiguousarray(xv[:, sl]),
            "wt": np.ascontiguousarray(wt[sl]),
        })
    res = run_bass_kernel_spmd(nc, in_maps, core_ids=list(range(NCORES)),
                               trace=TRACE)
    LAST_RESULT = res
    out = np.concatenate(
        [res.results[c]["out"].reshape(SPC, Cout, H, W) for c in range(NCORES)],
        axis=0)
    return np.ascontiguousarray(out.astype(np.float32))
